# revision 19
# baseline (speedup 1.0000x reference)
"""Pointer-Generator Network kernel for Trainium2 (8 NeuronCores, Bass/Tile).

Shapes (hardcoded): B=16, T=512, S=2048, D=768, fp32.

Math insight: scores[b,t,s] = dec_score[b,t] + enc_score[b,s] + b_ptr, and
softmax over s is invariant to the per-row additive constant dec_score[b,t]
(and b_ptr).  The masked positions are set to the constant -1e9, whose exp
underflows to exactly 0 in fp32 in the reference as well.  Therefore

    pointer_weights[b,t,s] = exp(enc_score[b,s]) * mask[b,s] / Z[b]   (no t dep!)
    context[b,t,:]         = (sum_s pm[b,s] * enc[b,s,:]) / Z[b]      (no t dep!)
    p_gen[b,t]             = sigmoid(dec[b,t,:]@g_d + context_row[b]@g_e + b_gen)

so the heavy [B,T,S] softmax and [B,T,S]x[B,S,D] einsum collapse to an [S]-sized
softmax and a [S]x[S,D] vector-matrix product per batch, plus broadcast writes.
The kernel is then purely DMA/HBM-bound (write pw/ctx, read enc/dec).

Sharding: data-parallel over batch, 2 batches per core, no collectives.
"""

import numpy as np
from contextlib import ExitStack

import concourse.bass as bass
import concourse.mybir as mybir
from concourse import bacc, tile
from concourse.masks import make_identity
from concourse.bass_utils import run_bass_kernel_spmd

B, T, S, D = 16, 512, 2048, 768
NCORES = 8
BPC = B // NCORES          # batches per core = 2
P = 128
NS = S // P                # 16 s-chunks
NT = T // P                # 4 t-chunks
F32 = mybir.dt.float32
I32 = mybir.dt.int32
MULT = mybir.AluOpType.mult
ADD = mybir.AluOpType.add
AXF = mybir.AxisListType.X
EXP = mybir.ActivationFunctionType.Exp
SIGMOID = mybir.ActivationFunctionType.Sigmoid
COPY = mybir.ActivationFunctionType.Copy

_CACHE = {}


def build_nc(loop_iters=None):
    nc = bacc.Bacc("TRN2", target_bir_lowering=False)
    enc_d = nc.declare_dram_parameter("enc", [BPC, S, D], F32, isOutput=False)
    dec_d = nc.declare_dram_parameter("dec", [BPC, T, D], F32, isOutput=False)
    mask_d = nc.declare_dram_parameter("mask", [BPC, S], I32, isOutput=False)
    we_d = nc.declare_dram_parameter("w_e", [D], F32, isOutput=False)
    gd_d = nc.declare_dram_parameter("g_d", [D], F32, isOutput=False)
    ge_d = nc.declare_dram_parameter("g_e", [D], F32, isOutput=False)
    bg_d = nc.declare_dram_parameter("b_gen", [1], F32, isOutput=False)
    pw_d = nc.declare_dram_parameter("pw", [BPC, T, S], F32, isOutput=True)
    pg_d = nc.declare_dram_parameter("pgen", [BPC, T, 1], F32, isOutput=True)
    cx_d = nc.declare_dram_parameter("ctx", [BPC, T, D], F32, isOutput=True)

    # N-dim column split for matmuls limited to 512 moving free dim
    DCOLS = [(0, 512), (512, 256)]

    with tile.TileContext(nc) as tc, ExitStack() as ectx:
        sb = ectx.enter_context(tc.tile_pool(name="sb", bufs=1))
        ps = ectx.enter_context(tc.tile_pool(name="ps", bufs=1, space="PSUM"))

        # ---- constants (bufs=1 tags: allocated once, live forever) ----
        identity = sb.tile([P, P], F32, tag="identity", name="identity")
        make_identity(nc, identity)
        ones_row = sb.tile([1, P], F32, tag="ones_row", name="ones_row")
        nc.vector.memset(ones_row, 1.0)
        ones_col = sb.tile([P, 1], F32, tag="ones_col", name="ones_col")
        nc.vector.memset(ones_col, 1.0)
        zbias = sb.tile([P, 1], F32, tag="zbias", name="zbias")
        nc.vector.memset(zbias, 0.0)

        we_row = sb.tile([1, D], F32, tag="we_row", name="we_row")
        nc.sync.dma_start(we_row, we_d[:].rearrange("(a d) -> a d", a=1))
        gd_row = sb.tile([1, D], F32, tag="gd_row", name="gd_row")
        nc.sync.dma_start(gd_row, gd_d[:].rearrange("(a d) -> a d", a=1))
        ge_row = sb.tile([1, D], F32, tag="ge_row", name="ge_row")
        nc.sync.dma_start(ge_row, ge_d[:].rearrange("(a d) -> a d", a=1))
        bgen_sb = sb.tile([1, 1], F32, tag="bgen", name="bgen_sb")
        nc.sync.dma_start(bgen_sb, bg_d[:].rearrange("(a d) -> a d", a=1))

        # broadcast w_e / g_d across 128 partitions via ones-matmul
        we_bc = sb.tile([P, D], F32, tag="we_bc", name="we_bc")
        gd_bc = sb.tile([P, D], F32, tag="gd_bc", name="gd_bc")
        for bi, (row, bc) in enumerate([(we_row, we_bc), (gd_row, gd_bc)]):
            for j, (c0, cn) in enumerate(DCOLS):
                pt = ps.tile([P, 512], F32, tag="pwb", bufs=2,
                             name=f"setup_bc_{bi}_{j}")
                nc.tensor.matmul(pt[:, :cn], ones_row, row[0:1, c0:c0 + cn],
                                 start=True, stop=True)
                nc.scalar.copy(bc[:, c0:c0 + cn], pt[:, :cn])

        from contextlib import nullcontext
        loop_cm = tc.For_i(0, loop_iters, 1) if loop_iters else nullcontext()
        with loop_cm:
            body(nc, tc, sb, ps, locals())

    nc.compile()
    return nc


def body(nc, tc, sb, ps, env):
    enc_d, dec_d, mask_d = env["enc_d"], env["dec_d"], env["mask_d"]
    pw_d, pg_d, cx_d = env["pw_d"], env["pg_d"], env["cx_d"]
    identity, ones_row, ones_col, zbias = (env["identity"], env["ones_row"],
                                           env["ones_col"], env["zbias"])
    we_bc, gd_bc, ge_row, bgen_sb = (env["we_bc"], env["gd_bc"],
                                     env["ge_row"], env["bgen_sb"])
    DCOLS = env["DCOLS"]
    if True:
        for b in range(BPC):
            # ---- load encoder tiles (s on partitions) ----
            enc_t = []
            for i in range(NS):
                et = sb.tile([P, D], F32, tag="enc", bufs=2 * NS,
                             name=f"enc_{b}_{i}")
                nc.sync.dma_start(et, enc_d[b, P * i:P * (i + 1), :])
                enc_t.append(et)

            # ---- enc_score: scores[:, i] = enc_tile_i @ w_e  (DVE fused) ----
            scores = sb.tile([P, NS], F32, tag="scores", bufs=2,
                             name=f"scores_{b}")
            # DVE does the elementwise product; the otherwise-idle ScalarE
            # does the row reduction via activation accum_out.
            for i in range(NS):
                jt = sb.tile([P, D], F32, tag="junk", bufs=5,
                             name=f"junk_{b}_{i}")
                nc.vector.tensor_tensor(out=jt, in0=enc_t[i], in1=we_bc, op=MULT)
                jt2 = sb.tile([P, D], F32, tag="junk", bufs=5,
                              name=f"junk2_{b}_{i}")
                nc.scalar.activation(jt2, jt, COPY,
                                     accum_out=scores[:, i:i + 1])

            # ---- mask: load [16,128] i32, cast f32, PE-transpose -> [128,16] ----
            mask_i = sb.tile([NS, P], I32, tag="mask_i", bufs=2,
                             name=f"mask_i_{b}")
            nc.sync.dma_start(mask_i, mask_d[b].rearrange("(i f) -> i f", i=NS))
            mask_f = sb.tile([NS, P], F32, tag="mask_f", bufs=2,
                             name=f"mask_f_{b}")
            nc.vector.tensor_copy(mask_f, mask_i)
            maskT_ps = ps.tile([P, NS], F32, tag="tp", bufs=2,
                               name=f"maskT_ps_{b}")
            nc.tensor.transpose(maskT_ps, mask_f, identity[:NS, :NS])

            # ---- pm = exp(scores) * mask; rowsum over free dim ----
            exp128 = sb.tile([P, NS], F32, tag="exp", bufs=2, name=f"exp_{b}")
            nc.scalar.activation(exp128, scores, EXP, bias=zbias)
            pm = sb.tile([P, NS], F32, tag="pm", bufs=2, name=f"pm_{b}")
            rowsum = sb.tile([P, 1], F32, tag="rowsum", bufs=2,
                             name=f"rowsum_{b}")
            nc.vector.tensor_tensor(out=pm, in0=exp128, in1=maskT_ps, op=MULT)
            nc.vector.tensor_reduce(out=rowsum, in_=pm, axis=AXF, op=ADD)

            # ---- Z = sum over partitions; rinv = 1/Z; broadcast to [128,1] ----
            z_ps = ps.tile([1, 1], F32, tag="zz", bufs=2, name=f"z_ps_{b}")
            nc.tensor.matmul(z_ps, rowsum, ones_col, start=True, stop=True)
            rinv = sb.tile([1, 1], F32, tag="rinv", bufs=2, name=f"rinv_{b}")
            nc.vector.reciprocal(rinv, z_ps)
            rb_ps = ps.tile([P, 1], F32, tag="zz", bufs=2, name=f"rb_ps_{b}")
            nc.tensor.matmul(rb_ps, ones_row, rinv, start=True, stop=True)
            rinv_bc = sb.tile([P, 1], F32, tag="rinv_bc", bufs=2,
                              name=f"rinv_bc_{b}")
            nc.scalar.copy(rinv_bc, rb_ps)

            # ---- p as a single row [1, 2048] (transpose + sbuf-to-sbuf dma) ----
            pmT_ps = ps.tile([NS, P], F32, tag="tp", bufs=2,
                             name=f"pmT_ps_{b}")
            nc.tensor.transpose(pmT_ps, pm, identity)
            pmT = sb.tile([NS, P], F32, tag="pmT", bufs=2, name=f"pmT_{b}")
            nc.scalar.copy(pmT, pmT_ps)
            p_row = sb.tile([1, S], F32, tag="p_row", bufs=2, name=f"p_row_{b}")
            nc.sync.dma_start(p_row.rearrange("a (i f) -> a i f", i=NS), pmT)

            # ---- pointer_weights: broadcast p_row to 128 partitions,
            #      normalize with rinv on ACT during PSUM->SBUF copy, DMA out ----
            pw_t = sb.tile([P, S], F32, tag="pw_t", bufs=2, name=f"pw_t_{b}")
            for j in range(S // 512):
                bc_ps = ps.tile([P, 512], F32, tag="pwb", bufs=2,
                                name=f"pw_bc_{b}_{j}")
                nc.tensor.matmul(bc_ps, ones_row, p_row[0:1, 512 * j:512 * (j + 1)],
                                 start=True, stop=True)
                nc.vector.tensor_scalar_mul(pw_t[:, 512 * j:512 * (j + 1)],
                                            bc_ps, rinv_bc)
            for k in range(NT):
                nc.sync.dma_start(pw_d[b, P * k:P * (k + 1), :], pw_t)

            # ---- context row: ctx_ps[1, D] = sum_i pm[:, i].T @ enc_tile_i ----
            ctx_ps = ps.tile([1, D], F32, tag="ctx", bufs=1, name=f"ctx_ps_{b}")
            for (c0, cn) in DCOLS:
                for i in range(NS):
                    nc.tensor.matmul(ctx_ps[:, c0:c0 + cn], pm[:, i:i + 1],
                                     enc_t[i][:, c0:c0 + cn],
                                     start=(i == 0), stop=(i == NS - 1))
            ctx_row = sb.tile([1, D], F32, tag="ctx_row", bufs=2,
                              name=f"ctx_row_{b}")
            nc.vector.tensor_scalar_mul(ctx_row, ctx_ps, rinv)

            # ---- context broadcast + DMA out ----
            ctx_t = sb.tile([P, D], F32, tag="ctx_t", bufs=2, name=f"ctx_t_{b}")
            for j, (c0, cn) in enumerate(DCOLS):
                cb_ps = ps.tile([P, 512], F32, tag="pwb", bufs=2,
                                name=f"cb_ps_{b}_{j}")
                nc.tensor.matmul(cb_ps[:, :cn], ones_row, ctx_row[0:1, c0:c0 + cn],
                                 start=True, stop=True)
                nc.scalar.copy(ctx_t[:, c0:c0 + cn], cb_ps[:, :cn])
            for k in range(NT):
                nc.sync.dma_start(cx_d[b, P * k:P * (k + 1), :], ctx_t)

            # ---- p_gen ----
            dsc = sb.tile([P, NT], F32, tag="dsc", bufs=2, name=f"dsc_{b}")
            for k in range(NT):
                dt_ = sb.tile([P, D], F32, tag="dec", bufs=2 * NT,
                              name=f"dec_{b}_{k}")
                nc.sync.dma_start(dt_, dec_d[b, P * k:P * (k + 1), :])
                jd = sb.tile([P, D], F32, tag="junk", bufs=5,
                             name=f"junkd_{b}_{k}")
                nc.vector.tensor_tensor(out=jd, in0=dt_, in1=gd_bc, op=MULT)
                jd2 = sb.tile([P, D], F32, tag="junk", bufs=5,
                              name=f"junkd2_{b}_{k}")
                nc.scalar.activation(jd2, jd, COPY,
                                     accum_out=dsc[:, k:k + 1])
            # c2 = ctx_row . g_e + b_gen, broadcast to [128, 1]
            jr = sb.tile([1, D], F32, tag="junk1", bufs=2, name=f"jr_{b}")
            jr2 = sb.tile([1, D], F32, tag="junk1", bufs=2, name=f"jr2_{b}")
            c2 = sb.tile([1, 1], F32, tag="c2", bufs=2, name=f"c2_{b}")
            nc.vector.tensor_tensor(out=jr, in0=ctx_row, in1=ge_row, op=MULT)
            nc.scalar.activation(jr2, jr, COPY, accum_out=c2)
            c2b = sb.tile([1, 1], F32, tag="c2b", bufs=2, name=f"c2b_{b}")
            nc.vector.tensor_add(c2b, c2, bgen_sb)
            gb_ps = ps.tile([P, 1], F32, tag="zz", bufs=2, name=f"gb_ps_{b}")
            nc.tensor.matmul(gb_ps, ones_row, c2b, start=True, stop=True)
            gb = sb.tile([P, 1], F32, tag="gb", bufs=2, name=f"gb_{b}")
            nc.scalar.copy(gb, gb_ps)
            pgen_t = sb.tile([P, NT], F32, tag="pgen_t", bufs=2,
                             name=f"pgen_t_{b}")
            nc.scalar.activation(pgen_t, dsc, SIGMOID, bias=gb)
            pgT_ps = ps.tile([NT, P], F32, tag="tp", bufs=2,
                             name=f"pgT_ps_{b}")
            nc.tensor.transpose(pgT_ps, pgen_t, identity)
            pgT = sb.tile([NT, P], F32, tag="pgT", bufs=2, name=f"pgT_{b}")
            nc.scalar.copy(pgT, pgT_ps)
            nc.sync.dma_start(
                pg_d[b].rearrange("(k f) a -> k (f a)", k=NT), pgT)


def _run(inputs, trace=False, trace_cores=None, loop_iters=None):
    key = ("nc", loop_iters)
    if key not in _CACHE:
        _CACHE[key] = build_nc(loop_iters)
    nc = _CACHE[key]

    dec = np.ascontiguousarray(np.asarray(inputs["decoder_hidden"], dtype=np.float32))
    enc = np.ascontiguousarray(np.asarray(inputs["encoder_outputs"], dtype=np.float32))
    mask = np.ascontiguousarray(np.asarray(inputs["encoder_mask"], dtype=np.int32))
    w_ptr = np.asarray(inputs["w_ptr"], dtype=np.float32)
    w_gen = np.asarray(inputs["w_gen"], dtype=np.float32)
    b_gen = np.ascontiguousarray(np.asarray(inputs["b_gen"], dtype=np.float32))
    w_e = np.ascontiguousarray(w_ptr[D:])
    g_d = np.ascontiguousarray(w_gen[:D])
    g_e = np.ascontiguousarray(w_gen[D:])

    in_maps = []
    for c in range(NCORES):
        lo, hi = BPC * c, BPC * (c + 1)
        in_maps.append({
            "enc": np.ascontiguousarray(enc[lo:hi]),
            "dec": np.ascontiguousarray(dec[lo:hi]),
            "mask": np.ascontiguousarray(mask[lo:hi]),
            "w_e": w_e, "g_d": g_d, "g_e": g_e, "b_gen": b_gen,
        })

    res = run_bass_kernel_spmd(nc, in_maps, list(range(NCORES)),
                               trace=trace, trace_cores=trace_cores)
    pw = np.concatenate([r["pw"] for r in res.results], axis=0)
    pgen = np.concatenate([r["pgen"] for r in res.results], axis=0)
    ctx = np.concatenate([r["ctx"] for r in res.results], axis=0)
    return (pw, pgen, ctx), res


def kernel(decoder_hidden, encoder_outputs, encoder_mask, w_ptr, b_ptr,
           w_gen, b_gen):
    outs, _ = _run({
        "decoder_hidden": decoder_hidden,
        "encoder_outputs": encoder_outputs,
        "encoder_mask": encoder_mask,
        "w_ptr": w_ptr, "b_ptr": b_ptr, "w_gen": w_gen, "b_gen": b_gen,
    })
    return outs


# revision 21
# speedup vs baseline: 3.5829x; 3.5829x over previous
"""Pointer-Generator Network kernel for Trainium2 (8 NeuronCores, Bass/Tile).

Shapes (hardcoded): B=16, T=512, S=2048, D=768, fp32.

Math insight: scores[b,t,s] = dec_score[b,t] + enc_score[b,s] + b_ptr, and
softmax over s is invariant to the per-row additive constant dec_score[b,t]
(and b_ptr).  The masked positions are set to the constant -1e9, whose exp
underflows to exactly 0 in fp32 in the reference as well.  Therefore

    pointer_weights[b,t,s] = exp(enc_score[b,s]) * mask[b,s] / Z[b]   (no t dep!)
    context[b,t,:]         = (sum_s pm[b,s] * enc[b,s,:]) / Z[b]      (no t dep!)
    p_gen[b,t]             = sigmoid(dec[b,t,:]@g_d + context_row[b]@g_e + b_gen)

so the heavy [B,T,S] softmax and [B,T,S]x[B,S,D] einsum collapse to an [S]-sized
softmax and a [S]x[S,D] vector-matrix product per batch, plus broadcast writes.
The kernel is then purely DMA/HBM-bound (write pw/ctx, read enc/dec).

Sharding: data-parallel over batch, 2 batches per core, no collectives.
"""

import numpy as np
from contextlib import ExitStack

import concourse.bass as bass
import concourse.mybir as mybir
from concourse import bacc, tile
from concourse.masks import make_identity
from concourse.bass_utils import run_bass_kernel_spmd

B, T, S, D = 16, 512, 2048, 768
NCORES = 8
BPC = B // NCORES          # batches per core = 2
P = 128
NS = S // P                # 16 s-chunks
NT = T // P                # 4 t-chunks
F32 = mybir.dt.float32
I32 = mybir.dt.int32
MULT = mybir.AluOpType.mult
ADD = mybir.AluOpType.add
AXF = mybir.AxisListType.X
EXP = mybir.ActivationFunctionType.Exp
SIGMOID = mybir.ActivationFunctionType.Sigmoid
COPY = mybir.ActivationFunctionType.Copy

_CACHE = {}


def build_nc(loop_iters=None, bench_io=False):
    nc = bacc.Bacc("TRN2", target_bir_lowering=False)
    if bench_io:
        # timing-only variant: big tensors live in internal DRAM so the
        # host<->device transfer noise disappears from wall-clock deltas
        enc_d = nc.dram_tensor("enc", [BPC, S, D], F32)
        dec_d = nc.dram_tensor("dec", [BPC, T, D], F32)
        mask_d = nc.dram_tensor("mask", [BPC, S], I32)
        we_d = nc.dram_tensor("w_e", [D], F32)
        gd_d = nc.dram_tensor("g_d", [D], F32)
        ge_d = nc.dram_tensor("g_e", [D], F32)
        bg_d = nc.dram_tensor("b_gen", [1], F32)
        pw_d = nc.dram_tensor("pw", [BPC, T, S], F32)
        pg_d = nc.dram_tensor("pgen", [BPC, T, 1], F32)
        cx_d = nc.dram_tensor("ctx", [BPC, T, D], F32)
        x_d = nc.declare_dram_parameter("x", [1, 16], F32, isOutput=False)
        y_d = nc.declare_dram_parameter("y", [1, 16], F32, isOutput=True)
    else:
        enc_d = nc.declare_dram_parameter("enc", [BPC, S, D], F32, isOutput=False)
        dec_d = nc.declare_dram_parameter("dec", [BPC, T, D], F32, isOutput=False)
        mask_d = nc.declare_dram_parameter("mask", [BPC, S], I32, isOutput=False)
        we_d = nc.declare_dram_parameter("w_e", [D], F32, isOutput=False)
        gd_d = nc.declare_dram_parameter("g_d", [D], F32, isOutput=False)
        ge_d = nc.declare_dram_parameter("g_e", [D], F32, isOutput=False)
        bg_d = nc.declare_dram_parameter("b_gen", [1], F32, isOutput=False)
        pw_d = nc.declare_dram_parameter("pw", [BPC, T, S], F32, isOutput=True)
        pg_d = nc.declare_dram_parameter("pgen", [BPC, T, 1], F32, isOutput=True)
        cx_d = nc.declare_dram_parameter("ctx", [BPC, T, D], F32, isOutput=True)

    # N-dim column split for matmuls limited to 512 moving free dim
    DCOLS = [(0, 512), (512, 256)]

    with tile.TileContext(nc) as tc, ExitStack() as ectx:
        sb = ectx.enter_context(tc.tile_pool(name="sb", bufs=1))
        ps = ectx.enter_context(tc.tile_pool(name="ps", bufs=1, space="PSUM"))

        # ---- constants (bufs=1 tags: allocated once, live forever) ----
        identity = sb.tile([P, P], F32, tag="identity", name="identity")
        make_identity(nc, identity)
        ones_row = sb.tile([1, P], F32, tag="ones_row", name="ones_row")
        nc.vector.memset(ones_row, 1.0)
        ones_col = sb.tile([P, 1], F32, tag="ones_col", name="ones_col")
        nc.vector.memset(ones_col, 1.0)
        zbias = sb.tile([P, 1], F32, tag="zbias", name="zbias")
        nc.vector.memset(zbias, 0.0)

        we_row = sb.tile([1, D], F32, tag="we_row", name="we_row")
        nc.sync.dma_start(we_row, we_d[:].rearrange("(a d) -> a d", a=1))
        gd_row = sb.tile([1, D], F32, tag="gd_row", name="gd_row")
        nc.sync.dma_start(gd_row, gd_d[:].rearrange("(a d) -> a d", a=1))
        ge_row = sb.tile([1, D], F32, tag="ge_row", name="ge_row")
        nc.sync.dma_start(ge_row, ge_d[:].rearrange("(a d) -> a d", a=1))
        bgen_sb = sb.tile([1, 1], F32, tag="bgen", name="bgen_sb")
        nc.sync.dma_start(bgen_sb, bg_d[:].rearrange("(a d) -> a d", a=1))

        # broadcast w_e / g_d across 128 partitions via ones-matmul
        we_bc = sb.tile([P, D], F32, tag="we_bc", name="we_bc")
        gd_bc = sb.tile([P, D], F32, tag="gd_bc", name="gd_bc")
        for bi, (row, bc) in enumerate([(we_row, we_bc), (gd_row, gd_bc)]):
            for j, (c0, cn) in enumerate(DCOLS):
                pt = ps.tile([P, 512], F32, tag="pwb", bufs=2,
                             name=f"setup_bc_{bi}_{j}")
                nc.tensor.matmul(pt[:, :cn], ones_row, row[0:1, c0:c0 + cn],
                                 start=True, stop=True)
                nc.scalar.copy(bc[:, c0:c0 + cn], pt[:, :cn])

        from contextlib import nullcontext
        if bench_io:
            xt = sb.tile([1, 16], F32, tag="xt", name="xt")
            nc.sync.dma_start(xt, x_d[:, :])
        loop_cm = tc.For_i(0, loop_iters, 1) if loop_iters else nullcontext()
        with loop_cm:
            body(nc, tc, sb, ps, locals())
        if bench_io:
            nc.sync.dma_start(y_d[:, :], xt)

    nc.compile()
    return nc


def body(nc, tc, sb, ps, env):
    enc_d, dec_d, mask_d = env["enc_d"], env["dec_d"], env["mask_d"]
    pw_d, pg_d, cx_d = env["pw_d"], env["pg_d"], env["cx_d"]
    identity, ones_row, ones_col, zbias = (env["identity"], env["ones_row"],
                                           env["ones_col"], env["zbias"])
    we_bc, gd_bc, ge_row, bgen_sb = (env["we_bc"], env["gd_bc"],
                                     env["ge_row"], env["bgen_sb"])
    DCOLS = env["DCOLS"]
    if True:
        for b in range(BPC):
            # ---- load encoder tiles (s on partitions) ----
            enc_t = []
            for i in range(NS):
                et = sb.tile([P, D], F32, tag="enc", bufs=2 * NS,
                             name=f"enc_{b}_{i}")
                nc.sync.dma_start(et, enc_d[b, P * i:P * (i + 1), :])
                enc_t.append(et)

            # ---- enc_score: scores[:, i] = enc_tile_i @ w_e  (DVE fused) ----
            scores = sb.tile([P, NS], F32, tag="scores", bufs=2,
                             name=f"scores_{b}")
            # DVE does the elementwise product; the otherwise-idle ScalarE
            # does the row reduction via activation accum_out.
            for i in range(NS):
                jt = sb.tile([P, D], F32, tag="junk", bufs=5,
                             name=f"junk_{b}_{i}")
                nc.vector.tensor_tensor(out=jt, in0=enc_t[i], in1=we_bc, op=MULT)
                jt2 = sb.tile([P, D], F32, tag="junk", bufs=5,
                              name=f"junk2_{b}_{i}")
                nc.scalar.activation(jt2, jt, COPY,
                                     accum_out=scores[:, i:i + 1])

            # ---- mask: load [16,128] i32, cast f32, PE-transpose -> [128,16] ----
            mask_i = sb.tile([NS, P], I32, tag="mask_i", bufs=2,
                             name=f"mask_i_{b}")
            nc.sync.dma_start(mask_i, mask_d[b].rearrange("(i f) -> i f", i=NS))
            mask_f = sb.tile([NS, P], F32, tag="mask_f", bufs=2,
                             name=f"mask_f_{b}")
            nc.vector.tensor_copy(mask_f, mask_i)
            maskT_ps = ps.tile([P, NS], F32, tag="tp", bufs=2,
                               name=f"maskT_ps_{b}")
            nc.tensor.transpose(maskT_ps, mask_f, identity[:NS, :NS])

            # ---- pm = exp(scores) * mask; rowsum over free dim ----
            exp128 = sb.tile([P, NS], F32, tag="exp", bufs=2, name=f"exp_{b}")
            nc.scalar.activation(exp128, scores, EXP, bias=zbias)
            pm = sb.tile([P, NS], F32, tag="pm", bufs=2, name=f"pm_{b}")
            rowsum = sb.tile([P, 1], F32, tag="rowsum", bufs=2,
                             name=f"rowsum_{b}")
            nc.vector.tensor_tensor(out=pm, in0=exp128, in1=maskT_ps, op=MULT)
            nc.vector.tensor_reduce(out=rowsum, in_=pm, axis=AXF, op=ADD)

            # ---- Z = sum over partitions; rinv = 1/Z; broadcast to [128,1] ----
            z_ps = ps.tile([1, 1], F32, tag="zz", bufs=2, name=f"z_ps_{b}")
            nc.tensor.matmul(z_ps, rowsum, ones_col, start=True, stop=True)
            rinv = sb.tile([1, 1], F32, tag="rinv", bufs=2, name=f"rinv_{b}")
            nc.vector.reciprocal(rinv, z_ps)
            rb_ps = ps.tile([P, 1], F32, tag="zz", bufs=2, name=f"rb_ps_{b}")
            nc.tensor.matmul(rb_ps, ones_row, rinv, start=True, stop=True)
            rinv_bc = sb.tile([P, 1], F32, tag="rinv_bc", bufs=2,
                              name=f"rinv_bc_{b}")
            nc.scalar.copy(rinv_bc, rb_ps)

            # ---- p as a single row [1, 2048] (transpose + sbuf-to-sbuf dma) ----
            pmT_ps = ps.tile([NS, P], F32, tag="tp", bufs=2,
                             name=f"pmT_ps_{b}")
            nc.tensor.transpose(pmT_ps, pm, identity)
            pmT = sb.tile([NS, P], F32, tag="pmT", bufs=2, name=f"pmT_{b}")
            nc.scalar.copy(pmT, pmT_ps)
            p_row = sb.tile([1, S], F32, tag="p_row", bufs=2, name=f"p_row_{b}")
            nc.sync.dma_start(p_row.rearrange("a (i f) -> a i f", i=NS), pmT)

            # ---- pointer_weights: broadcast p_row to 128 partitions,
            #      normalize with rinv on ACT during PSUM->SBUF copy, DMA out ----
            pw_t = sb.tile([P, S], F32, tag="pw_t", bufs=2, name=f"pw_t_{b}")
            for j in range(S // 512):
                bc_ps = ps.tile([P, 512], F32, tag="pwb", bufs=2,
                                name=f"pw_bc_{b}_{j}")
                nc.tensor.matmul(bc_ps, ones_row, p_row[0:1, 512 * j:512 * (j + 1)],
                                 start=True, stop=True)
                nc.vector.tensor_scalar_mul(pw_t[:, 512 * j:512 * (j + 1)],
                                            bc_ps, rinv_bc)
            for k in range(NT):
                nc.sync.dma_start(pw_d[b, P * k:P * (k + 1), :], pw_t)

            # ---- context row: ctx_ps[1, D] = sum_i pm[:, i].T @ enc_tile_i ----
            ctx_ps = ps.tile([1, D], F32, tag="ctx", bufs=1, name=f"ctx_ps_{b}")
            for (c0, cn) in DCOLS:
                for i in range(NS):
                    nc.tensor.matmul(ctx_ps[:, c0:c0 + cn], pm[:, i:i + 1],
                                     enc_t[i][:, c0:c0 + cn],
                                     start=(i == 0), stop=(i == NS - 1))
            ctx_row = sb.tile([1, D], F32, tag="ctx_row", bufs=2,
                              name=f"ctx_row_{b}")
            nc.vector.tensor_scalar_mul(ctx_row, ctx_ps, rinv)

            # ---- context broadcast + DMA out ----
            ctx_t = sb.tile([P, D], F32, tag="ctx_t", bufs=2, name=f"ctx_t_{b}")
            for j, (c0, cn) in enumerate(DCOLS):
                cb_ps = ps.tile([P, 512], F32, tag="pwb", bufs=2,
                                name=f"cb_ps_{b}_{j}")
                nc.tensor.matmul(cb_ps[:, :cn], ones_row, ctx_row[0:1, c0:c0 + cn],
                                 start=True, stop=True)
                nc.scalar.copy(ctx_t[:, c0:c0 + cn], cb_ps[:, :cn])
            for k in range(NT):
                nc.sync.dma_start(cx_d[b, P * k:P * (k + 1), :], ctx_t)

            # ---- p_gen ----
            dsc = sb.tile([P, NT], F32, tag="dsc", bufs=2, name=f"dsc_{b}")
            for k in range(NT):
                dt_ = sb.tile([P, D], F32, tag="dec", bufs=2 * NT,
                              name=f"dec_{b}_{k}")
                nc.sync.dma_start(dt_, dec_d[b, P * k:P * (k + 1), :])
                jd = sb.tile([P, D], F32, tag="junk", bufs=5,
                             name=f"junkd_{b}_{k}")
                nc.vector.tensor_tensor(out=jd, in0=dt_, in1=gd_bc, op=MULT)
                jd2 = sb.tile([P, D], F32, tag="junk", bufs=5,
                              name=f"junkd2_{b}_{k}")
                nc.scalar.activation(jd2, jd, COPY,
                                     accum_out=dsc[:, k:k + 1])
            # c2 = ctx_row . g_e + b_gen, broadcast to [128, 1]
            jr = sb.tile([1, D], F32, tag="junk1", bufs=2, name=f"jr_{b}")
            jr2 = sb.tile([1, D], F32, tag="junk1", bufs=2, name=f"jr2_{b}")
            c2 = sb.tile([1, 1], F32, tag="c2", bufs=2, name=f"c2_{b}")
            nc.vector.tensor_tensor(out=jr, in0=ctx_row, in1=ge_row, op=MULT)
            nc.scalar.activation(jr2, jr, COPY, accum_out=c2)
            c2b = sb.tile([1, 1], F32, tag="c2b", bufs=2, name=f"c2b_{b}")
            nc.vector.tensor_add(c2b, c2, bgen_sb)
            gb_ps = ps.tile([P, 1], F32, tag="zz", bufs=2, name=f"gb_ps_{b}")
            nc.tensor.matmul(gb_ps, ones_row, c2b, start=True, stop=True)
            gb = sb.tile([P, 1], F32, tag="gb", bufs=2, name=f"gb_{b}")
            nc.scalar.copy(gb, gb_ps)
            pgen_t = sb.tile([P, NT], F32, tag="pgen_t", bufs=2,
                             name=f"pgen_t_{b}")
            nc.scalar.activation(pgen_t, dsc, SIGMOID, bias=gb)
            pgT_ps = ps.tile([NT, P], F32, tag="tp", bufs=2,
                             name=f"pgT_ps_{b}")
            nc.tensor.transpose(pgT_ps, pgen_t, identity)
            pgT = sb.tile([NT, P], F32, tag="pgT", bufs=2, name=f"pgT_{b}")
            nc.scalar.copy(pgT, pgT_ps)
            nc.sync.dma_start(
                pg_d[b].rearrange("(k f) a -> k (f a)", k=NT), pgT)


def _run(inputs, trace=False, trace_cores=None, loop_iters=None):
    key = ("nc", loop_iters)
    if key not in _CACHE:
        _CACHE[key] = build_nc(loop_iters)
    nc = _CACHE[key]

    dec = np.ascontiguousarray(np.asarray(inputs["decoder_hidden"], dtype=np.float32))
    enc = np.ascontiguousarray(np.asarray(inputs["encoder_outputs"], dtype=np.float32))
    mask = np.ascontiguousarray(np.asarray(inputs["encoder_mask"], dtype=np.int32))
    w_ptr = np.asarray(inputs["w_ptr"], dtype=np.float32)
    w_gen = np.asarray(inputs["w_gen"], dtype=np.float32)
    b_gen = np.ascontiguousarray(np.asarray(inputs["b_gen"], dtype=np.float32))
    w_e = np.ascontiguousarray(w_ptr[D:])
    g_d = np.ascontiguousarray(w_gen[:D])
    g_e = np.ascontiguousarray(w_gen[D:])

    in_maps = []
    for c in range(NCORES):
        lo, hi = BPC * c, BPC * (c + 1)
        in_maps.append({
            "enc": np.ascontiguousarray(enc[lo:hi]),
            "dec": np.ascontiguousarray(dec[lo:hi]),
            "mask": np.ascontiguousarray(mask[lo:hi]),
            "w_e": w_e, "g_d": g_d, "g_e": g_e, "b_gen": b_gen,
        })

    res = run_bass_kernel_spmd(nc, in_maps, list(range(NCORES)),
                               trace=trace, trace_cores=trace_cores)
    pw = np.concatenate([r["pw"] for r in res.results], axis=0)
    pgen = np.concatenate([r["pgen"] for r in res.results], axis=0)
    ctx = np.concatenate([r["ctx"] for r in res.results], axis=0)
    return (pw, pgen, ctx), res


def kernel(decoder_hidden, encoder_outputs, encoder_mask, w_ptr, b_ptr,
           w_gen, b_gen):
    outs, _ = _run({
        "decoder_hidden": decoder_hidden,
        "encoder_outputs": encoder_outputs,
        "encoder_mask": encoder_mask,
        "w_ptr": w_ptr, "b_ptr": b_ptr, "w_gen": w_gen, "b_gen": b_gen,
    })
    return outs
